# revision 2
# baseline (speedup 1.0000x reference)
"""Trainium2 Bass kernel for nn_AsymmetricProjectedLinear (8 NeuronCores).

Reference computes out = x @ W_large^T with
    W_large = (A_out @ B_out) @ W_small @ (A_in @ B_in)^T    [4096, 4096]

W_large (137 GFLOP naive) is never materialized. Factored (~4.5 GFLOP):
    M   = B_in @ W_small^T @ B_out^T            [64, 64]   (tiny)
    out = ((x @ A_in) @ M) @ A_out^T            [4096t, 4096]

Sharding: tokens (B*S = 4096) split 512/core across 8 cores; weights
replicated. Host work is layout-only (transpose/pack/slice/dtype-cast);
all FLOPs run on the NeuronCores.

v2 design (from perfetto analysis of the 53.2us v1):
  - v1's wire schedule serialized: W_small (2MB) rode ahead of x on the
    two HWDGE rings, x(B0) was not resident until ~24us, the tile
    scheduler pushed the prework-M chain to ~30us, and the first output
    byte left at ~38us with a 5us DMA hole at 31-36us.
  - v2 splits W_small across sync/scalar HWDGE AND the gpsimd SWDGE
    queue (which v1 left idle until the tail), so rings reach x(B0)
    ~4us sooner and G can chase w pieces on three fronts.
  - prework (G -> M) is emitted BEFORE stage1 so the priority-heap
    scheduler finishes M as soon as w lands instead of burying the
    transpose chain behind stage1(B1).
  - stage5(B0) is emitted BEFORE stage1(B1): in v1 the o-slices were
    interleaved into the x(B1) chase, so every B0 output slice waited
    on x(B1) DMA it did not depend on.
  - outputs leave in per-2-slice 0.5MB chunk-events round-robined over
    all three queues (gpsimd first - it is free mid-kernel; ring
    chunks queue FIFO behind x(B1) and flow right after it).
  - stage5 slice order 0,1,4,5,2,3,6,7 chases the a_outT quarters in
    their two-ring arrival order.
  - bf16 everywhere (rel err ~5.5e-3 vs the 2e-2 harness gate); PSUM
    stays fp32 (TRN2 matmul cannot write 16-bit PSUM).
  - Column-packed stage1 (even k-tiles -> PE column strip 0, odd ->
    strip 1), m_q = [[M,M],[M,M]] K-folds the strips in stage2, and
    stage5 row-packs the two 128-token halves - unchanged from v1.
"""

import numpy as np

import concourse.bass as bass
import concourse.mybir as mybir
import concourse.tile as tile
from concourse import bacc
from concourse.bass_utils import run_bass_kernel_spmd

N_CORES = 8
Bsz, S, D = 2, 2048, 4096
TOK = Bsz * S          # 4096 tokens
T = TOK // N_CORES     # 512 tokens per core
TB = 256               # tokens per stage-1 block
NBLK = T // TB         # 2 blocks
RANK = 64
DS = 1024              # d_small
KT = D // 128          # 32 k-tiles over d_in_large

F32 = mybir.dt.float32
BF16 = mybir.dt.bfloat16
OUT_DT = BF16

# wsm packed columns: b_outT | b_inT | a_in | ident
C_BOT = 0
C_BIT = C_BOT + 8 * RANK
C_AIN = C_BIT + 8 * RANK
C_IDT = C_AIN + KT * RANK
C_WSM = C_IDT + RANK
C_SPLIT = C_AIN + 9 * RANK   # ring split boundary (1600 / 1536 cols)

# stage5 slice order: chase a_outT quarters (sync: q0,q1; scalar: q2,q3)
S5_ORDER = [0, 1, 4, 5, 2, 3, 6, 7]

_nc_cache = {}


def build():
    if "nc" in _nc_cache:
        return _nc_cache["nc"]
    nc = bacc.Bacc("TRN2", target_bir_lowering=False, debug=False,
                   num_devices=N_CORES)

    # x_p: [NBLK, 4 pieces, 2 rings, 128, 4 k-tiles * TB]
    x_p = nc.dram_tensor("x_p", [NBLK, 4, 2, 128, 4 * TB], BF16,
                         kind="ExternalInput")
    wsm_p = nc.dram_tensor("wsm_p", [128, C_WSM], BF16, kind="ExternalInput")
    w_p = nc.dram_tensor("w_p", [128, 8 * DS], BF16, kind="ExternalInput")
    # a_outT_p: [128, D] with rows 0-63 and 64-127 both = A_out^T
    a_outT_p = nc.dram_tensor("a_outT_p", [128, D], BF16, kind="ExternalInput")
    out = nc.dram_tensor("out", [T, D], OUT_DT, kind="ExternalOutput")

    with tile.TileContext(nc) as tc:
        with (
            tc.tile_pool(name="const", bufs=1) as cpool,
            tc.tile_pool(name="xin", bufs=2) as xpool,
            tc.tile_pool(name="outp", bufs=2) as opool,
            tc.tile_pool(name="interm", bufs=2) as ipool,
            tc.tile_pool(name="ps_u", bufs=1, space="PSUM") as ps_u,
            tc.tile_pool(name="ps_g", bufs=1, space="PSUM") as ps_g,
            tc.tile_pool(name="ps_gt", bufs=1, space="PSUM") as ps_gt,
            tc.tile_pool(name="ps_t2", bufs=1, space="PSUM") as ps_t2,
            tc.tile_pool(name="ps_mp", bufs=1, space="PSUM") as ps_mp,
            tc.tile_pool(name="ps_o", bufs=3, space="PSUM") as ps_o,
        ):
            rings = (nc.sync, nc.scalar)

            # ---- constant tiles --------------------------------------
            wsm_s = cpool.tile([128, C_WSM], BF16)
            b_outT_s = wsm_s[:, C_BOT:C_BOT + 8 * RANK]
            b_inT_s = wsm_s[:, C_BIT:C_BIT + 8 * RANK]
            a_in_s = wsm_s[:, C_AIN:C_AIN + KT * RANK]
            # identity replicated in both partition halves (host side)
            ident_h = (wsm_s[:RANK, C_IDT:C_IDT + RANK],
                       wsm_s[RANK:128, C_IDT:C_IDT + RANK])
            w_tile = cpool.tile([128, 8 * DS], BF16)
            a_outT_s = cpool.tile([128, D], BF16)
            jnk_s = cpool.tile([128, 576], BF16)
            x_tiles = [[None] * 8 for _ in range(NBLK)]

            # ---- ring emission (FIFO order = stream order) -----------
            def dma_wsm():
                rings[0].dma_start(out=wsm_s[:, :C_SPLIT],
                                   in_=wsm_p.ap()[:, :C_SPLIT])
                rings[1].dma_start(out=wsm_s[:, C_SPLIT:],
                                   in_=wsm_p.ap()[:, C_SPLIT:])

            def dma_w():
                # W_small j-tiles: j0,j1 -> sync ring; j2,j3 -> scalar
                # ring; j4..j7 -> gpsimd SWDGE (idle until the tail in
                # v1).  G's jorder chases this arrival order.
                for j in range(2):
                    sl = slice(j * DS, (j + 1) * DS)
                    rings[0].dma_start(out=w_tile[:, sl], in_=w_p.ap()[:, sl])
                for j in range(2, 4):
                    sl = slice(j * DS, (j + 1) * DS)
                    rings[1].dma_start(out=w_tile[:, sl], in_=w_p.ap()[:, sl])
                for j in range(4, 8):
                    sl = slice(j * DS, (j + 1) * DS)
                    nc.gpsimd.dma_start(out=w_tile[:, sl], in_=w_p.ap()[:, sl])

            def dma_x(b):
                for p in range(4):
                    for r in range(2):
                        xt = xpool.tile([128, 4 * TB], BF16,
                                        tag=f"x{p * 2 + r}")
                        rings[r].dma_start(out=xt[:, :],
                                           in_=x_p.ap()[b, p, r, :, :])
                        x_tiles[b][p * 2 + r] = xt

            def dma_a_outT():
                # quarters: sync q0,q1; scalar q2,q3 (S5_ORDER chases)
                for h in range(2):
                    for r in range(2):
                        sl = slice((r * 2 + h) * 1024, (r * 2 + h + 1) * 1024)
                        rings[r].dma_start(out=a_outT_s[:, sl],
                                           in_=a_outT_p.ap()[:, sl])

            # out chunk-events: (engine for o_t0 chunk, engine for o_t1)
            OUTQ = [
                (nc.gpsimd, nc.gpsimd),   # b0 sl 0,1
                (nc.gpsimd, nc.sync),     # b0 sl 4,5
                (nc.scalar, nc.gpsimd),   # b0 sl 2,3
                (nc.sync, nc.scalar),     # b0 sl 6,7
                (nc.gpsimd, nc.sync),     # b1 sl 0,1
                (nc.scalar, nc.gpsimd),   # b1 sl 4,5
                (nc.sync, nc.scalar),     # b1 sl 2,3
                (nc.gpsimd, nc.sync),     # b1 sl 6,7
            ]

            def dma_out_chunk(b, o_t0, o_t1, o_lo, ev):
                # one 0.25MB chunk per tile covering slices o_lo,o_lo+1
                r0 = b * TB
                sl = slice(o_lo * 512, (o_lo + 2) * 512)
                e0, e1 = OUTQ[ev]
                e0.dma_start(out=out.ap()[r0:r0 + 128, sl], in_=o_t0[:, sl])
                e1.dma_start(out=out.ap()[r0 + 128:r0 + TB, sl],
                             in_=o_t1[:, sl])

            # ---- compute ---------------------------------------------
            def warmup():
                # PE HAM needs ~3.4us of sustained busy to lift the
                # clock gate; burn junk matmuls on memset data while
                # the first DMAs stream. Results are discarded.
                nc.gpsimd.memset(jnk_s[:, :], 0.0)
                for i in range(7):
                    jp = ps_t2.tile([RANK, 512], F32, tag="t2ps")
                    nc.tensor.matmul(jp[:, :], jnk_s[:, 0:RANK],
                                     jnk_s[:, 64:576], start=True, stop=True)

            def prework_g():
                # G = B_out @ W_small [64, DS], column-packed over the
                # two 512-col halves: h=0 lands on PSUM partitions
                # 0-63, h=1 on 64-127 (concurrent PE column strips,
                # one bank). jorder matches the three-queue arrival:
                # sync j0,j1; scalar j2,j3; gpsimd j4..j7.
                g_ps = ps_g.tile([128, 512], F32, tag="ps_g")
                jorder = [0, 2, 4, 1, 3, 5, 6, 7]
                for idx, j in enumerate(jorder):
                    for h in range(2):
                        nc.tensor.matmul(
                            g_ps[h * RANK:(h + 1) * RANK, :],
                            b_outT_s[:, j * RANK:(j + 1) * RANK],
                            w_tile[:, j * DS + h * 512:j * DS + (h + 1) * 512],
                            start=(idx == 0), stop=(idx == 7),
                        )
                g_s = ipool.tile([128, 512], BF16, tag="g")
                nc.vector.tensor_copy(g_s[:, :], g_ps[:, :])
                return g_s

            def prework_m(g_s):
                # Move G's h=1 half (PSUM partitions 64-127) down to
                # 0-63 with one identity matmul (PE transposes only
                # support base-0 inputs), then G^T via PE transpose (4
                # chunks per half into one PSUM tile, one drain copy),
                # then M = B_in @ G^T, computed twice via PE column
                # strips so M lands on partitions 0-63 AND 64-127.
                g2_ps = ps_g.tile([RANK, 512], F32, tag="ps_g")
                nc.tensor.matmul(g2_ps[:, :], ident_h[1][:, :],
                                 g_s[RANK:128, :], start=True, stop=True)
                g2_s = ipool.tile([RANK, 512], BF16, tag="g2")
                nc.vector.tensor_copy(g2_s[:, :], g2_ps[:, :])
                gT_s = ipool.tile([128, 8 * RANK], BF16, tag="gT")
                gt_ps = ps_gt.tile([128, 8 * RANK], BF16, tag="gt")
                for it in range(8):
                    h, c = it // 4, it % 4
                    src = g_s if h == 0 else g2_s
                    nc.tensor.transpose(
                        gt_ps[:, it * RANK:(it + 1) * RANK],
                        src[0:RANK, c * 128:(c + 1) * 128],
                        ident_h[0][:, :])
                nc.vector.tensor_copy(gT_s[:, :], gt_ps[:, :])
                m_ps = ps_mp.tile([128, RANK], F32, tag="mps")
                for it in range(8):
                    for half in range(2):
                        nc.tensor.matmul(
                            m_ps[half * RANK:(half + 1) * RANK, :],
                            b_inT_s[:, it * RANK:(it + 1) * RANK],
                            gT_s[:, it * RANK:(it + 1) * RANK],
                            start=(it == 0), stop=(it == 7),
                        )
                # m_q [128, 128] = [[M, M], [M, M]]: K=128 folds the
                # two stage-1 column-strip partial sums; duplicated
                # cols land t2 in both PSUM halves.
                m_q = ipool.tile([128, 128], BF16, tag="mq")
                nc.vector.tensor_copy(m_q[:, 0:RANK], m_ps[:, :])
                nc.vector.tensor_copy(m_q[:, RANK:128], m_ps[:, :])
                return m_q

            def stage1(b):
                # column-packed: even k-tiles -> PSUM partitions 0-63,
                # odd k-tiles -> partitions 64-127 (concurrent strips).
                u1 = ps_u.tile([128, TB], F32, tag="u1")
                for i in range(KT // 2):
                    me, mo = 2 * i, 2 * i + 1
                    xt = x_tiles[b][me // 4]
                    ke, ko = me % 4, mo % 4
                    nc.tensor.matmul(
                        u1[0:RANK, :],
                        a_in_s[:, me * RANK:(me + 1) * RANK],
                        xt[:, ke * TB:(ke + 1) * TB],
                        start=(i == 0), stop=(i == KT // 2 - 1),
                    )
                    nc.tensor.matmul(
                        u1[RANK:128, :],
                        a_in_s[:, mo * RANK:(mo + 1) * RANK],
                        xt[:, ko * TB:(ko + 1) * TB],
                        start=(i == 0), stop=(i == KT // 2 - 1),
                    )
                u1_s = ipool.tile([128, TB], BF16, tag="u1s")
                nc.vector.tensor_copy(u1_s[:, :], u1[:, :])
                return u1_s

            def stage2(u1_s, m_q):
                t2_ps = ps_t2.tile([128, TB], F32, tag="t2ps")
                nc.tensor.matmul(t2_ps[:, :], m_q[:, :], u1_s[:, :],
                                 start=True, stop=True)
                t2_s = ipool.tile([128, TB], BF16, tag="t2s")
                nc.vector.tensor_copy(t2_s[:, :], t2_ps[:, :])
                return t2_s

            def stage5_slice(t2_s, o_t0, o_t1, o):
                sl = slice(o * 512, (o + 1) * 512)
                po0 = ps_o.tile([128, 512], F32, tag="ps_out")
                po1 = ps_o.tile([128, 512], F32, tag="ps_out")
                nc.tensor.matmul(
                    po0[:, :], t2_s[0:RANK, 0:128],
                    a_outT_s[0:RANK, sl], start=True, stop=True)
                nc.tensor.matmul(
                    po1[:, :], t2_s[RANK:128, 128:TB],
                    a_outT_s[RANK:128, sl], start=True, stop=True)
                nc.vector.tensor_copy(o_t0[:, sl], po0[:, :])
                nc.scalar.copy(o_t1[:, sl], po1[:, :])

            def stage5_block(b, t2_s, ev0):
                o_t0 = opool.tile([128, D], OUT_DT, tag="ot0")
                o_t1 = opool.tile([128, D], OUT_DT, tag="ot1")
                for k, o in enumerate(S5_ORDER):
                    stage5_slice(t2_s, o_t0, o_t1, o)
                    if k % 2 == 1:
                        dma_out_chunk(b, o_t0, o_t1, S5_ORDER[k - 1],
                                      ev0 + k // 2)

            # ---- emission (order = scheduler priority) ---------------
            dma_wsm()
            dma_w()
            dma_x(0)
            dma_a_outT()
            dma_x(1)

            warmup()
            g_s = prework_g()
            m_q = prework_m(g_s)
            u1_b0 = stage1(0)
            t2_b0 = stage2(u1_b0, m_q)
            stage5_block(0, t2_b0, ev0=0)
            u1_b1 = stage1(1)
            t2_b1 = stage2(u1_b1, m_q)
            stage5_block(1, t2_b1, ev0=4)

    nc.compile()
    _nc_cache["nc"] = nc
    return nc


def _prep_in_maps(x, W_small, A_out, B_out, A_in, B_in):
    import ml_dtypes
    f = ml_dtypes.bfloat16
    x2 = np.asarray(x, dtype=f).reshape(TOK, D)
    a_in_p = np.ascontiguousarray(
        np.asarray(A_in, f).reshape(KT, 128, RANK).transpose(1, 0, 2)
    ).reshape(128, KT * RANK)
    b_inT_p = np.ascontiguousarray(
        np.asarray(B_in, f).T.reshape(8, 128, RANK).transpose(1, 0, 2)
    ).reshape(128, 8 * RANK)
    b_outT_p = np.ascontiguousarray(
        np.asarray(B_out, f).T.reshape(8, 128, RANK).transpose(1, 0, 2)
    ).reshape(128, 8 * RANK)
    ident = np.zeros((128, RANK), f)
    ident[:RANK] = np.eye(RANK, dtype=f)
    ident[RANK:] = np.eye(RANK, dtype=f)
    wsm_p = np.ascontiguousarray(
        np.concatenate([b_outT_p, b_inT_p, a_in_p, ident], axis=1))
    w_p = np.ascontiguousarray(
        np.asarray(W_small, f).reshape(8, 128, DS).transpose(1, 0, 2)
    ).reshape(128, 8 * DS)
    aoT = np.asarray(A_out, f).T                     # [64, D]
    a_outT_p = np.ascontiguousarray(np.concatenate([aoT, aoT], axis=0))

    shared = {"wsm_p": wsm_p, "w_p": w_p, "a_outT_p": a_outT_p}
    in_maps = []
    for c in range(N_CORES):
        xs = x2[c * T:(c + 1) * T, :]                # [T, D]
        # chunk (B, p, r): tokens [B*TB,(B+1)*TB), k-tiles p*8+r*4 ..+4
        xp = np.ascontiguousarray(
            xs.T                                     # [D, T]
            .reshape(4, 2, 4, 128, NBLK, TB)         # p, r, kk, part, B, t
            .transpose(4, 0, 1, 3, 2, 5)             # B, p, r, part, kk, t
        ).reshape(NBLK, 4, 2, 128, 4 * TB)
        in_maps.append({"x_p": xp, **shared})
    return in_maps


def _run(inputs, trace=False):
    nc = build()
    in_maps = _prep_in_maps(**inputs)
    res = run_bass_kernel_spmd(
        nc, in_maps, core_ids=list(range(N_CORES)), trace=trace
    )
    out = np.concatenate(
        [np.asarray(res.results[c]["out"], dtype=np.float32)
         for c in range(N_CORES)], axis=0
    ).reshape(Bsz, S, D)
    return out, res


def kernel(**inputs) -> np.ndarray:
    out, _ = _run(inputs, trace=False)
    return out


# revision 12
# speedup vs baseline: 1.1240x; 1.1240x over previous
"""Trainium2 Bass kernel for nn_AsymmetricProjectedLinear (8 NeuronCores).

Reference computes out = x @ W_large^T with
    W_large = (A_out @ B_out) @ W_small @ (A_in @ B_in)^T    [4096, 4096]

W_large (137 GFLOP naive) is never materialized. Factored (~4.5 GFLOP):
    M   = B_in @ W_small^T @ B_out^T            [64, 64]   (tiny)
    out = ((x @ A_in) @ M) @ A_out^T            [4096t, 4096]

Sharding: tokens (B*S = 4096) split 512/core across 8 cores; weights
replicated. Host work is layout-only (transpose/pack/slice/dtype-cast);
all FLOPs run on the NeuronCores.

v2 design (from perfetto analysis of the 53.2us v1):
  - v1's wire schedule serialized: W_small (2MB) rode ahead of x on the
    two HWDGE rings, x(B0) was not resident until ~24us, the tile
    scheduler pushed the prework-M chain to ~30us, and the first output
    byte left at ~38us with a 5us DMA hole at 31-36us.
  - v2 splits W_small across sync/scalar HWDGE AND the gpsimd SWDGE
    queue (which v1 left idle until the tail), so rings reach x(B0)
    ~4us sooner and G can chase w pieces on three fronts.
  - prework (G -> M) is emitted BEFORE stage1 so the priority-heap
    scheduler finishes M as soon as w lands instead of burying the
    transpose chain behind stage1(B1).
  - stage5(B0) is emitted BEFORE stage1(B1): in v1 the o-slices were
    interleaved into the x(B1) chase, so every B0 output slice waited
    on x(B1) DMA it did not depend on.
  - outputs leave in per-2-slice 0.5MB chunk-events round-robined over
    all three queues (gpsimd first - it is free mid-kernel; ring
    chunks queue FIFO behind x(B1) and flow right after it).
  - stage5 slice order 0,1,4,5,2,3,6,7 chases the a_outT quarters in
    their two-ring arrival order.
  - bf16 everywhere (rel err ~5.5e-3 vs the 2e-2 harness gate); PSUM
    stays fp32 (TRN2 matmul cannot write 16-bit PSUM).
  - Column-packed stage1 (even k-tiles -> PE column strip 0, odd ->
    strip 1), m_q = [[M,M],[M,M]] K-folds the strips in stage2, and
    stage5 row-packs the two 128-token halves - unchanged from v1.
"""

import numpy as np

import concourse.bass as bass
import concourse.mybir as mybir
import concourse.tile as tile
from concourse import bacc
from concourse.bass_utils import run_bass_kernel_spmd

N_CORES = 8
Bsz, S, D = 2, 2048, 4096
TOK = Bsz * S          # 4096 tokens
T = TOK // N_CORES     # 512 tokens per core
TB = 256               # tokens per stage-1 block
NBLK = T // TB         # 2 blocks
RANK = 64
DS = 1024              # d_small
KT = D // 128          # 32 k-tiles over d_in_large

F32 = mybir.dt.float32
BF16 = mybir.dt.bfloat16
OUT_DT = BF16

# wsm packed columns: b_outT | b_inT2 (column-duplicated) | a_in | ident
C_BOT = 0
C_BIT = C_BOT + 8 * RANK
C_AIN = C_BIT + 8 * 2 * RANK
C_IDT = C_AIN + KT * RANK
C_WSM = C_IDT + RANK
C_SPLIT = C_WSM // 2         # ring split boundary

# stage5 slice order: chase a_outT quarters (sync: q0,q1; scalar: q2,q3)
S5_ORDER = [0, 1, 4, 5, 2, 3, 6, 7]

_nc_cache = {}


def build():
    if "nc" in _nc_cache:
        return _nc_cache["nc"]
    nc = bacc.Bacc("TRN2", target_bir_lowering=False, debug=False,
                   num_devices=N_CORES)

    # x_p: [NBLK, 4 pieces, 2 rings, 128, 4 k-tiles * TB]
    x_p = nc.dram_tensor("x_p", [NBLK, 4, 2, 128, 4 * TB], BF16,
                         kind="ExternalInput")
    wsm_p = nc.dram_tensor("wsm_p", [128, C_WSM], BF16, kind="ExternalInput")
    w_p = nc.dram_tensor("w_p", [128, 8 * DS], BF16, kind="ExternalInput")
    # a_outT_p: [128, D] with rows 0-63 and 64-127 both = A_out^T
    a_outT_p = nc.dram_tensor("a_outT_p", [128, D], BF16, kind="ExternalInput")
    out = nc.dram_tensor("out", [T, D], OUT_DT, kind="ExternalOutput")

    with tile.TileContext(nc) as tc:
        with (
            tc.tile_pool(name="const", bufs=1) as cpool,
            tc.tile_pool(name="xin", bufs=2) as xpool,
            tc.tile_pool(name="outp", bufs=2) as opool,
            tc.tile_pool(name="interm", bufs=2) as ipool,
            tc.tile_pool(name="ps_u", bufs=1, space="PSUM") as ps_u,
            tc.tile_pool(name="ps_g", bufs=1, space="PSUM") as ps_g,
            tc.tile_pool(name="ps_gt", bufs=1, space="PSUM") as ps_gt,
            tc.tile_pool(name="ps_t2", bufs=1, space="PSUM") as ps_t2,
            tc.tile_pool(name="ps_mp", bufs=1, space="PSUM") as ps_mp,
            tc.tile_pool(name="ps_o", bufs=3, space="PSUM") as ps_o,
        ):
            rings = (nc.sync, nc.scalar)

            # ---- constant tiles --------------------------------------
            wsm_s = cpool.tile([128, C_WSM], BF16)
            b_outT_s = wsm_s[:, C_BOT:C_BOT + 8 * RANK]
            b_inT2_s = wsm_s[:, C_BIT:C_BIT + 8 * 2 * RANK]
            a_in_s = wsm_s[:, C_AIN:C_AIN + KT * RANK]
            # identity replicated in both partition halves (host side)
            ident_h = (wsm_s[:RANK, C_IDT:C_IDT + RANK],
                       wsm_s[RANK:128, C_IDT:C_IDT + RANK])
            w_tile = cpool.tile([128, 8 * DS], BF16)
            a_outT_s = cpool.tile([128, D], BF16)
            x_tiles = [[None] * 8 for _ in range(NBLK)]

            # ---- ring emission (FIFO order = stream order) -----------
            def dma_wsm():
                rings[0].dma_start(out=wsm_s[:, :C_SPLIT],
                                   in_=wsm_p.ap()[:, :C_SPLIT])
                rings[1].dma_start(out=wsm_s[:, C_SPLIT:],
                                   in_=wsm_p.ap()[:, C_SPLIT:])

            def dma_w():
                # 0.5MB pieces on the two HWDGE rings only. SWDGE input
                # traffic concurrent with ring activity cannibalizes
                # aggregate bandwidth (v2 measured ~290GB/s for three
                # queues vs ~400GB/s for two rings alone).
                for h in range(2):
                    for r in range(2):
                        sl = slice((r * 2 + h) * 2 * DS,
                                   (r * 2 + h + 1) * 2 * DS)
                        rings[r].dma_start(out=w_tile[:, sl],
                                           in_=w_p.ap()[:, sl])

            def dma_x(b):
                for p in range(4):
                    for r in range(2):
                        xt = xpool.tile([128, 4 * TB], BF16,
                                        tag=f"x{p * 2 + r}")
                        rings[r].dma_start(out=xt[:, :],
                                           in_=x_p.ap()[b, p, r, :, :])
                        x_tiles[b][p * 2 + r] = xt

            def dma_a_outT():
                # quarters: sync q0,q1; scalar q2,q3 (S5_ORDER chases)
                for h in range(2):
                    for r in range(2):
                        sl = slice((r * 2 + h) * 1024, (r * 2 + h + 1) * 1024)
                        rings[r].dma_start(out=a_outT_s[:, sl],
                                           in_=a_outT_p.ap()[:, sl])

            # out chunk-events: (engine for o_t0 chunk, engine for o_t1)
            # gpsimd (SWDGE q0) is free from the start; ring chunks
            # queue FIFO behind the remaining inputs. scalar is kept
            # light in the tail - it is the ACTIVATE drain engine.
            OUTQ = [
                (nc.gpsimd, nc.gpsimd),   # b0 sl 0,1
                (nc.gpsimd, nc.scalar),   # b0 sl 4,5
                (nc.scalar, nc.gpsimd),   # b0 sl 2,3
                (nc.sync, nc.gpsimd),     # b0 sl 6,7
                (nc.gpsimd, nc.sync),     # b1 sl 0,1
                (nc.sync, nc.gpsimd),     # b1 sl 4,5
                (nc.gpsimd, nc.sync),     # b1 sl 2,3
                (nc.sync, nc.gpsimd),     # b1 sl 6,7
            ]

            def dma_out_chunk(b, o_t0, o_t1, o_lo, ev):
                # one 0.25MB chunk per tile covering slices o_lo,o_lo+1
                r0 = b * TB
                sl = slice(o_lo * 512, (o_lo + 2) * 512)
                e0, e1 = OUTQ[ev]
                e0.dma_start(out=out.ap()[r0:r0 + 128, sl], in_=o_t0[:, sl])
                e1.dma_start(out=out.ap()[r0 + 128:r0 + TB, sl],
                             in_=o_t1[:, sl])

            # ---- compute ---------------------------------------------
            def prework_g():
                # G = B_out @ W_small [64, DS], column-packed over the
                # two 512-col halves: h=0 lands on PSUM partitions
                # 0-63, h=1 on 64-127 (concurrent PE column strips,
                # one bank). j interleaved to match the two rings'
                # arrival order (sync j0-3, scalar j4-7).
                g_ps = ps_g.tile([128, 512], F32, tag="ps_g")
                jorder = [0, 4, 1, 5, 2, 6, 3, 7]
                for idx, j in enumerate(jorder):
                    for h in range(2):
                        nc.tensor.matmul(
                            g_ps[h * RANK:(h + 1) * RANK, :],
                            b_outT_s[:, j * RANK:(j + 1) * RANK],
                            w_tile[:, j * DS + h * 512:j * DS + (h + 1) * 512],
                            start=(idx == 0), stop=(idx == 7),
                        )
                g_s = ipool.tile([128, 512], BF16, tag="g")
                nc.vector.tensor_copy(g_s[:, :], g_ps[:, :])
                return g_s

            def prework_m(g_s):
                # Move G's h=1 half (PSUM partitions 64-127) down to
                # 0-63 with one identity matmul (PE transposes only
                # support base-0 inputs), then G^T via PE transpose (4
                # chunks per half into one PSUM tile, one drain copy),
                # then M = B_in @ G^T, computed twice via PE column
                # strips so M lands on partitions 0-63 AND 64-127.
                g2_ps = ps_g.tile([RANK, 512], F32, tag="ps_g")
                nc.tensor.matmul(g2_ps[:, :], ident_h[1][:, :],
                                 g_s[RANK:128, :], start=True, stop=True)
                g2_s = ipool.tile([RANK, 512], BF16, tag="g2")
                nc.vector.tensor_copy(g2_s[:, :], g2_ps[:, :])
                gT_s = ipool.tile([128, 8 * RANK], BF16, tag="gT")
                gt_ps = ps_gt.tile([128, 8 * RANK], BF16, tag="gt")
                for it in range(8):
                    h, c = it // 4, it % 4
                    src = g_s if h == 0 else g2_s
                    nc.tensor.transpose(
                        gt_ps[:, it * RANK:(it + 1) * RANK],
                        src[0:RANK, c * 128:(c + 1) * 128],
                        ident_h[0][:, :])
                nc.vector.tensor_copy(gT_s[:, :], gt_ps[:, :])
                # b_inT2 holds [B_in^T tile | B_in^T tile] (128 wide),
                # so one matmul lands M on partitions 0-63 AND 64-127
                # (half the LDWEIGHTS of two column-strip matmuls).
                m_ps = ps_mp.tile([128, RANK], F32, tag="mps")
                for it in range(8):
                    nc.tensor.matmul(
                        m_ps[:, :],
                        b_inT2_s[:, it * 2 * RANK:(it + 1) * 2 * RANK],
                        gT_s[:, it * RANK:(it + 1) * RANK],
                        start=(it == 0), stop=(it == 7),
                    )
                # m_q [128, 128] = [[M, M], [M, M]]: K=128 folds the
                # two stage-1 column-strip partial sums; duplicated
                # cols land t2 in both PSUM halves.
                m_q = ipool.tile([128, 128], BF16, tag="mq")
                nc.vector.tensor_copy(m_q[:, 0:RANK], m_ps[:, :])
                nc.vector.tensor_copy(m_q[:, RANK:128], m_ps[:, :])
                return m_q

            def stage1(b):
                # column-packed: even k-tiles -> PSUM partitions 0-63,
                # odd k-tiles -> partitions 64-127 (concurrent strips).
                u1 = ps_u.tile([128, TB], F32, tag="u1")
                for i in range(KT // 2):
                    me, mo = 2 * i, 2 * i + 1
                    xt = x_tiles[b][me // 4]
                    ke, ko = me % 4, mo % 4
                    nc.tensor.matmul(
                        u1[0:RANK, :],
                        a_in_s[:, me * RANK:(me + 1) * RANK],
                        xt[:, ke * TB:(ke + 1) * TB],
                        start=(i == 0), stop=(i == KT // 2 - 1),
                    )
                    nc.tensor.matmul(
                        u1[RANK:128, :],
                        a_in_s[:, mo * RANK:(mo + 1) * RANK],
                        xt[:, ko * TB:(ko + 1) * TB],
                        start=(i == 0), stop=(i == KT // 2 - 1),
                    )
                u1_s = ipool.tile([128, TB], BF16, tag="u1s")
                nc.vector.tensor_copy(u1_s[:, :], u1[:, :])
                return u1_s

            def stage2(u1_s, m_q):
                t2_ps = ps_t2.tile([128, TB], F32, tag="t2ps")
                nc.tensor.matmul(t2_ps[:, :], m_q[:, :], u1_s[:, :],
                                 start=True, stop=True)
                t2_s = ipool.tile([128, TB], BF16, tag="t2s")
                nc.vector.tensor_copy(t2_s[:, :], t2_ps[:, :])
                return t2_s

            def stage5_slice(t2_s, o_t0, o_t1, o):
                sl = slice(o * 512, (o + 1) * 512)
                po0 = ps_o.tile([128, 512], F32, tag="ps_out")
                po1 = ps_o.tile([128, 512], F32, tag="ps_out")
                nc.tensor.matmul(
                    po0[:, :], t2_s[0:RANK, 0:128],
                    a_outT_s[0:RANK, sl], start=True, stop=True)
                nc.tensor.matmul(
                    po1[:, :], t2_s[RANK:128, 128:TB],
                    a_outT_s[RANK:128, sl], start=True, stop=True)
                nc.vector.tensor_copy(o_t0[:, sl], po0[:, :])
                nc.scalar.copy(o_t1[:, sl], po1[:, :])

            def stage5_block(b, t2_s, ev0):
                o_t0 = opool.tile([128, D], OUT_DT, tag="ot0")
                o_t1 = opool.tile([128, D], OUT_DT, tag="ot1")
                for k, o in enumerate(S5_ORDER):
                    stage5_slice(t2_s, o_t0, o_t1, o)
                    if k % 2 == 1:
                        dma_out_chunk(b, o_t0, o_t1, S5_ORDER[k - 1],
                                      ev0 + k // 2)

            # ---- emission (order = scheduler priority) ---------------
            dma_wsm()
            dma_w()
            dma_x(0)
            dma_a_outT()
            dma_x(1)

            g_s = prework_g()
            m_q = prework_m(g_s)
            u1_b0 = stage1(0)
            t2_b0 = stage2(u1_b0, m_q)
            stage5_block(0, t2_b0, ev0=0)
            u1_b1 = stage1(1)
            t2_b1 = stage2(u1_b1, m_q)
            stage5_block(1, t2_b1, ev0=4)

    nc.compile()
    _nc_cache["nc"] = nc
    return nc


def _prep_in_maps(x, W_small, A_out, B_out, A_in, B_in):
    import ml_dtypes
    f = ml_dtypes.bfloat16
    x2 = np.asarray(x, dtype=f).reshape(TOK, D)
    a_in_p = np.ascontiguousarray(
        np.asarray(A_in, f).reshape(KT, 128, RANK).transpose(1, 0, 2)
    ).reshape(128, KT * RANK)
    b_inT = np.asarray(B_in, f).T.reshape(8, 128, RANK).transpose(1, 0, 2)
    # duplicate columns: [tile | tile] per 128-row d-tile
    b_inT2_p = np.ascontiguousarray(
        np.concatenate([b_inT, b_inT], axis=2)
    ).reshape(128, 8 * 2 * RANK)
    b_outT_p = np.ascontiguousarray(
        np.asarray(B_out, f).T.reshape(8, 128, RANK).transpose(1, 0, 2)
    ).reshape(128, 8 * RANK)
    ident = np.zeros((128, RANK), f)
    ident[:RANK] = np.eye(RANK, dtype=f)
    ident[RANK:] = np.eye(RANK, dtype=f)
    wsm_p = np.ascontiguousarray(
        np.concatenate([b_outT_p, b_inT2_p, a_in_p, ident], axis=1))
    w_p = np.ascontiguousarray(
        np.asarray(W_small, f).reshape(8, 128, DS).transpose(1, 0, 2)
    ).reshape(128, 8 * DS)
    aoT = np.asarray(A_out, f).T                     # [64, D]
    a_outT_p = np.ascontiguousarray(np.concatenate([aoT, aoT], axis=0))

    shared = {"wsm_p": wsm_p, "w_p": w_p, "a_outT_p": a_outT_p}
    in_maps = []
    for c in range(N_CORES):
        xs = x2[c * T:(c + 1) * T, :]                # [T, D]
        # chunk (B, p, r): tokens [B*TB,(B+1)*TB), k-tiles p*8+r*4 ..+4
        xp = np.ascontiguousarray(
            xs.T                                     # [D, T]
            .reshape(4, 2, 4, 128, NBLK, TB)         # p, r, kk, part, B, t
            .transpose(4, 0, 1, 3, 2, 5)             # B, p, r, part, kk, t
        ).reshape(NBLK, 4, 2, 128, 4 * TB)
        in_maps.append({"x_p": xp, **shared})
    return in_maps


def _run(inputs, trace=False):
    nc = build()
    in_maps = _prep_in_maps(**inputs)
    res = run_bass_kernel_spmd(
        nc, in_maps, core_ids=list(range(N_CORES)), trace=trace
    )
    out = np.concatenate(
        [np.asarray(res.results[c]["out"], dtype=np.float32)
         for c in range(N_CORES)], axis=0
    ).reshape(Bsz, S, D)
    return out, res


def kernel(**inputs) -> np.ndarray:
    out, _ = _run(inputs, trace=False)
    return out


# revision 14
# speedup vs baseline: 1.2197x; 1.0852x over previous
"""Trainium2 Bass kernel for nn_AsymmetricProjectedLinear (8 NeuronCores).

Reference computes out = x @ W_large^T with
    W_large = (A_out @ B_out) @ W_small @ (A_in @ B_in)^T    [4096, 4096]

W_large (137 GFLOP naive) is never materialized. Factored (~4.5 GFLOP):
    M   = B_in @ W_small^T @ B_out^T            [64, 64]   (tiny)
    out = ((x @ A_in) @ M) @ A_out^T            [4096t, 4096]

Sharding: tokens (B*S = 4096) split 512/core across 8 cores; weights
replicated. Host work is layout-only (transpose/pack/slice/dtype-cast);
all FLOPs run on the NeuronCores.

v2 design (from perfetto analysis of the 53.2us v1):
  - v1's wire schedule serialized: W_small (2MB) rode ahead of x on the
    two HWDGE rings, x(B0) was not resident until ~24us, the tile
    scheduler pushed the prework-M chain to ~30us, and the first output
    byte left at ~38us with a 5us DMA hole at 31-36us.
  - v2 splits W_small across sync/scalar HWDGE AND the gpsimd SWDGE
    queue (which v1 left idle until the tail), so rings reach x(B0)
    ~4us sooner and G can chase w pieces on three fronts.
  - prework (G -> M) is emitted BEFORE stage1 so the priority-heap
    scheduler finishes M as soon as w lands instead of burying the
    transpose chain behind stage1(B1).
  - stage5(B0) is emitted BEFORE stage1(B1): in v1 the o-slices were
    interleaved into the x(B1) chase, so every B0 output slice waited
    on x(B1) DMA it did not depend on.
  - outputs leave in per-2-slice 0.5MB chunk-events round-robined over
    all three queues (gpsimd first - it is free mid-kernel; ring
    chunks queue FIFO behind x(B1) and flow right after it).
  - stage5 slice order 0,1,4,5,2,3,6,7 chases the a_outT quarters in
    their two-ring arrival order.
  - bf16 everywhere (rel err ~5.5e-3 vs the 2e-2 harness gate); PSUM
    stays fp32 (TRN2 matmul cannot write 16-bit PSUM).
  - Column-packed stage1 (even k-tiles -> PE column strip 0, odd ->
    strip 1), m_q = [[M,M],[M,M]] K-folds the strips in stage2, and
    stage5 row-packs the two 128-token halves - unchanged from v1.
"""

import numpy as np

import concourse.bass as bass
import concourse.mybir as mybir
import concourse.tile as tile
from concourse import bacc
from concourse.bass_utils import run_bass_kernel_spmd

N_CORES = 8
Bsz, S, D = 2, 2048, 4096
TOK = Bsz * S          # 4096 tokens
T = TOK // N_CORES     # 512 tokens per core
TB = 256               # tokens per stage-1 block
NBLK = T // TB         # 2 blocks
RANK = 64
DS = 1024              # d_small
KT = D // 128          # 32 k-tiles over d_in_large

F32 = mybir.dt.float32
BF16 = mybir.dt.bfloat16
OUT_DT = BF16

# wsm packed columns: b_outT | b_inT | a_in | ident
C_BOT = 0
C_BIT = C_BOT + 8 * RANK
C_AIN = C_BIT + 8 * RANK
C_IDT = C_AIN + KT * RANK
C_WSM = C_IDT + RANK
C_SPLIT = C_AIN + 9 * RANK   # ring split boundary (1600 / 1536 cols)

# stage5 slice order: chase a_outT quarters (sync: q0,q1; scalar: q2,q3)
S5_ORDER = [0, 1, 2, 3, 4, 5, 6, 7]

_nc_cache = {}


def build():
    if "nc" in _nc_cache:
        return _nc_cache["nc"]
    nc = bacc.Bacc("TRN2", target_bir_lowering=False, debug=False,
                   num_devices=N_CORES)

    # x_p: [NBLK, 4 pieces, 2 rings, 128, 4 k-tiles * TB]
    x_p = nc.dram_tensor("x_p", [NBLK, 4, 2, 128, 4 * TB], BF16,
                         kind="ExternalInput")
    wsm_p = nc.dram_tensor("wsm_p", [128, C_WSM], BF16, kind="ExternalInput")
    w_p = nc.dram_tensor("w_p", [128, 8 * DS], BF16, kind="ExternalInput")
    # a_outT_p: [128, D] with rows 0-63 and 64-127 both = A_out^T
    a_outT_p = nc.dram_tensor("a_outT_p", [128, D], BF16, kind="ExternalInput")
    out = nc.dram_tensor("out", [T, D], OUT_DT, kind="ExternalOutput")

    with tile.TileContext(nc) as tc:
        with (
            tc.tile_pool(name="const", bufs=1) as cpool,
            tc.tile_pool(name="xin", bufs=2) as xpool,
            tc.tile_pool(name="outp", bufs=2) as opool,
            tc.tile_pool(name="interm", bufs=2) as ipool,
            tc.tile_pool(name="ps_u", bufs=1, space="PSUM") as ps_u,
            tc.tile_pool(name="ps_g", bufs=1, space="PSUM") as ps_g,
            tc.tile_pool(name="ps_t2", bufs=1, space="PSUM") as ps_t2,
            tc.tile_pool(name="ps_mp", bufs=1, space="PSUM") as ps_mp,
            tc.tile_pool(name="ps_o", bufs=4, space="PSUM") as ps_o,
        ):
            rings = (nc.sync, nc.scalar)

            # ---- constant tiles --------------------------------------
            wsm_s = cpool.tile([128, C_WSM], BF16)
            b_outT_s = wsm_s[:, C_BOT:C_BOT + 8 * RANK]
            b_inT_s = wsm_s[:, C_BIT:C_BIT + 8 * RANK]
            a_in_s = wsm_s[:, C_AIN:C_AIN + KT * RANK]
            # identity replicated in both partition halves (host side)
            ident_h = (wsm_s[:RANK, C_IDT:C_IDT + RANK],
                       wsm_s[RANK:128, C_IDT:C_IDT + RANK])
            w_tile = cpool.tile([128, 8 * DS], BF16)
            a_outT_s = cpool.tile([128, D], BF16)
            x_tiles = [[None] * 8 for _ in range(NBLK)]

            # ---- ring emission (FIFO order = stream order) -----------
            def dma_wsm():
                rings[0].dma_start(out=wsm_s[:, :C_SPLIT],
                                   in_=wsm_p.ap()[:, :C_SPLIT])
                rings[1].dma_start(out=wsm_s[:, C_SPLIT:],
                                   in_=wsm_p.ap()[:, C_SPLIT:])

            def dma_w():
                # 0.5MB pieces on the two HWDGE rings only. SWDGE input
                # traffic concurrent with ring activity cannibalizes
                # aggregate bandwidth (v2 measured ~290GB/s for three
                # queues vs ~400GB/s for two rings alone).
                for h in range(2):
                    for r in range(2):
                        sl = slice((r * 2 + h) * 2 * DS,
                                   (r * 2 + h + 1) * 2 * DS)
                        rings[r].dma_start(out=w_tile[:, sl],
                                           in_=w_p.ap()[:, sl])

            def dma_x(b):
                for p in range(4):
                    for r in range(2):
                        xt = xpool.tile([128, 4 * TB], BF16,
                                        tag=f"x{p * 2 + r}")
                        rings[r].dma_start(out=xt[:, :],
                                           in_=x_p.ap()[b, p, r, :, :])
                        x_tiles[b][p * 2 + r] = xt

            def dma_a_outT():
                # quarters: sync q0,q1; scalar q2,q3 (S5_ORDER chases)
                for h in range(2):
                    for r in range(2):
                        sl = slice((r * 2 + h) * 1024, (r * 2 + h + 1) * 1024)
                        rings[r].dma_start(out=a_outT_s[:, sl],
                                           in_=a_outT_p.ap()[:, sl])

            # out chunk-events: (engine for o_t0 chunk, engine for o_t1)
            # gpsimd (SWDGE q0) is free from the start; ring chunks
            # queue FIFO behind the remaining inputs. scalar is kept
            # light in the tail - it is the ACTIVATE drain engine.
            OUTQ = [
                (nc.gpsimd, nc.gpsimd),   # b0 sl 0,1
                (nc.gpsimd, nc.scalar),   # b0 sl 4,5
                (nc.scalar, nc.gpsimd),   # b0 sl 2,3
                (nc.sync, nc.gpsimd),     # b0 sl 6,7
                (nc.gpsimd, nc.sync),     # b1 sl 0,1
                (nc.sync, nc.gpsimd),     # b1 sl 4,5
                (nc.gpsimd, nc.sync),     # b1 sl 2,3
                (nc.sync, nc.gpsimd),     # b1 sl 6,7
            ]

            def dma_out_chunk(b, o_t0, o_t1, o_lo, ev):
                # one 0.25MB chunk per tile covering slices o_lo,o_lo+1
                r0 = b * TB
                sl = slice(o_lo * 512, (o_lo + 2) * 512)
                e0, e1 = OUTQ[ev]
                e0.dma_start(out=out.ap()[r0:r0 + 128, sl], in_=o_t0[:, sl])
                e1.dma_start(out=out.ap()[r0 + 128:r0 + TB, sl],
                             in_=o_t1[:, sl])

            # ---- compute ---------------------------------------------
            def prework_g():
                # G = B_out @ W_small [64, DS], column-packed over the
                # two 512-col halves: h=0 lands on PSUM partitions
                # 0-63, h=1 on 64-127 (concurrent PE column strips,
                # one bank). j interleaved to match the two rings'
                # arrival order (sync j0-3, scalar j4-7).
                g_ps = ps_g.tile([128, 512], F32, tag="ps_g")
                jorder = [0, 4, 1, 5, 2, 6, 3, 7]
                for idx, j in enumerate(jorder):
                    for h in range(2):
                        nc.tensor.matmul(
                            g_ps[h * RANK:(h + 1) * RANK, :],
                            b_outT_s[:, j * RANK:(j + 1) * RANK],
                            w_tile[:, j * DS + h * 512:j * DS + (h + 1) * 512],
                            start=(idx == 0), stop=(idx == 7),
                        )
                g_s = ipool.tile([128, 512], BF16, tag="g")
                nc.vector.tensor_copy(g_s[:, :], g_ps[:, :])
                return g_s

            def prework_m(g_s):
                # Move G's h=1 half (PSUM partitions 64-127) down to
                # 0-63 with one identity matmul (PE transposes only
                # support base-0 inputs), then G^T via PE transpose (4
                # chunks per half into one PSUM tile, one drain copy),
                # then M = B_in @ G^T, computed twice via PE column
                # strips so M lands on partitions 0-63 AND 64-127.
                g2_ps = ps_g.tile([RANK, 512], F32, tag="ps_g")
                nc.tensor.matmul(g2_ps[:, :], ident_h[1][:, :],
                                 g_s[RANK:128, :], start=True, stop=True)
                g2_s = ipool.tile([RANK, 512], BF16, tag="g2")
                nc.vector.tensor_copy(g2_s[:, :], g2_ps[:, :])
                gT_s = ipool.tile([128, 8 * RANK], BF16, tag="gT")
                # reuses the ps_g slot (g/g2 are drained by now)
                gt_ps = ps_g.tile([128, 8 * RANK], BF16, tag="ps_g")
                for it in range(8):
                    h, c = it // 4, it % 4
                    src = g_s if h == 0 else g2_s
                    nc.tensor.transpose(
                        gt_ps[:, it * RANK:(it + 1) * RANK],
                        src[0:RANK, c * 128:(c + 1) * 128],
                        ident_h[0][:, :])
                nc.vector.tensor_copy(gT_s[:, :], gt_ps[:, :])
                m_ps = ps_mp.tile([128, RANK], F32, tag="mps")
                for it in range(8):
                    for half in range(2):
                        nc.tensor.matmul(
                            m_ps[half * RANK:(half + 1) * RANK, :],
                            b_inT_s[:, it * RANK:(it + 1) * RANK],
                            gT_s[:, it * RANK:(it + 1) * RANK],
                            start=(it == 0), stop=(it == 7),
                        )
                # m_q [128, 128] = [[M, M], [M, M]]: K=128 folds the
                # two stage-1 column-strip partial sums; duplicated
                # cols land t2 in both PSUM halves.
                m_q = ipool.tile([128, 128], BF16, tag="mq")
                nc.vector.tensor_copy(m_q[:, 0:RANK], m_ps[:, :])
                nc.vector.tensor_copy(m_q[:, RANK:128], m_ps[:, :])
                return m_q

            def stage1(b):
                # column-packed: even k-tiles -> PSUM partitions 0-63,
                # odd k-tiles -> partitions 64-127 (concurrent strips).
                u1 = ps_u.tile([128, TB], F32, tag="u1")
                for i in range(KT // 2):
                    me, mo = 2 * i, 2 * i + 1
                    xt = x_tiles[b][me // 4]
                    ke, ko = me % 4, mo % 4
                    nc.tensor.matmul(
                        u1[0:RANK, :],
                        a_in_s[:, me * RANK:(me + 1) * RANK],
                        xt[:, ke * TB:(ke + 1) * TB],
                        start=(i == 0), stop=(i == KT // 2 - 1),
                    )
                    nc.tensor.matmul(
                        u1[RANK:128, :],
                        a_in_s[:, mo * RANK:(mo + 1) * RANK],
                        xt[:, ko * TB:(ko + 1) * TB],
                        start=(i == 0), stop=(i == KT // 2 - 1),
                    )
                u1_s = ipool.tile([128, TB], BF16, tag="u1s")
                nc.vector.tensor_copy(u1_s[:, :], u1[:, :])
                return u1_s

            def stage2(u1_s, m_q):
                t2_ps = ps_t2.tile([128, TB], F32, tag="t2ps")
                nc.tensor.matmul(t2_ps[:, :], m_q[:, :], u1_s[:, :],
                                 start=True, stop=True)
                t2_s = ipool.tile([128, TB], BF16, tag="t2s")
                nc.vector.tensor_copy(t2_s[:, :], t2_ps[:, :])
                return t2_s

            def stage5_slice(t2_s, o_t0, o_t1, o):
                sl = slice(o * 512, (o + 1) * 512)
                po0 = ps_o.tile([128, 512], F32, tag="ps_out")
                po1 = ps_o.tile([128, 512], F32, tag="ps_out")
                nc.tensor.matmul(
                    po0[:, :], t2_s[0:RANK, 0:128],
                    a_outT_s[0:RANK, sl], start=True, stop=True)
                nc.tensor.matmul(
                    po1[:, :], t2_s[RANK:128, 128:TB],
                    a_outT_s[RANK:128, sl], start=True, stop=True)
                nc.vector.tensor_copy(o_t0[:, sl], po0[:, :])
                nc.scalar.copy(o_t1[:, sl], po1[:, :])

            def stage5_block(b, t2_s, ev0):
                o_t0 = opool.tile([128, D], OUT_DT, tag="ot0")
                o_t1 = opool.tile([128, D], OUT_DT, tag="ot1")
                for k, o in enumerate(S5_ORDER):
                    stage5_slice(t2_s, o_t0, o_t1, o)
                    if k % 2 == 1:
                        dma_out_chunk(b, o_t0, o_t1, S5_ORDER[k - 1],
                                      ev0 + k // 2)

            # ---- emission (order = scheduler priority) ---------------
            dma_wsm()
            dma_w()
            dma_a_outT()
            dma_x(0)
            dma_x(1)

            g_s = prework_g()
            m_q = prework_m(g_s)
            u1_b0 = stage1(0)
            t2_b0 = stage2(u1_b0, m_q)
            stage5_block(0, t2_b0, ev0=0)
            u1_b1 = stage1(1)
            t2_b1 = stage2(u1_b1, m_q)
            stage5_block(1, t2_b1, ev0=4)

    nc.compile()
    _nc_cache["nc"] = nc
    return nc


def _prep_in_maps(x, W_small, A_out, B_out, A_in, B_in):
    import ml_dtypes
    f = ml_dtypes.bfloat16
    x2 = np.asarray(x, dtype=f).reshape(TOK, D)
    a_in_p = np.ascontiguousarray(
        np.asarray(A_in, f).reshape(KT, 128, RANK).transpose(1, 0, 2)
    ).reshape(128, KT * RANK)
    b_inT_p = np.ascontiguousarray(
        np.asarray(B_in, f).T.reshape(8, 128, RANK).transpose(1, 0, 2)
    ).reshape(128, 8 * RANK)
    b_outT_p = np.ascontiguousarray(
        np.asarray(B_out, f).T.reshape(8, 128, RANK).transpose(1, 0, 2)
    ).reshape(128, 8 * RANK)
    ident = np.zeros((128, RANK), f)
    ident[:RANK] = np.eye(RANK, dtype=f)
    ident[RANK:] = np.eye(RANK, dtype=f)
    wsm_p = np.ascontiguousarray(
        np.concatenate([b_outT_p, b_inT_p, a_in_p, ident], axis=1))
    w_p = np.ascontiguousarray(
        np.asarray(W_small, f).reshape(8, 128, DS).transpose(1, 0, 2)
    ).reshape(128, 8 * DS)
    aoT = np.asarray(A_out, f).T                     # [64, D]
    a_outT_p = np.ascontiguousarray(np.concatenate([aoT, aoT], axis=0))

    shared = {"wsm_p": wsm_p, "w_p": w_p, "a_outT_p": a_outT_p}
    in_maps = []
    for c in range(N_CORES):
        xs = x2[c * T:(c + 1) * T, :]                # [T, D]
        # chunk (B, p, r): tokens [B*TB,(B+1)*TB), k-tiles p*8+r*4 ..+4
        xp = np.ascontiguousarray(
            xs.T                                     # [D, T]
            .reshape(4, 2, 4, 128, NBLK, TB)         # p, r, kk, part, B, t
            .transpose(4, 0, 1, 3, 2, 5)             # B, p, r, part, kk, t
        ).reshape(NBLK, 4, 2, 128, 4 * TB)
        in_maps.append({"x_p": xp, **shared})
    return in_maps


def _run(inputs, trace=False):
    nc = build()
    in_maps = _prep_in_maps(**inputs)
    res = run_bass_kernel_spmd(
        nc, in_maps, core_ids=list(range(N_CORES)), trace=trace
    )
    out = np.concatenate(
        [np.asarray(res.results[c]["out"], dtype=np.float32)
         for c in range(N_CORES)], axis=0
    ).reshape(Bsz, S, D)
    return out, res


def kernel(**inputs) -> np.ndarray:
    out, _ = _run(inputs, trace=False)
    return out
